# revision 35
# baseline (speedup 1.0000x reference)
"""2-layer GAT (DGL GATConv-style) on 8 Trainium2 NeuronCores.

v2 strategy (dst-sharded, matmul-direct layer 1):
  - Nodes are partitioned into 8 ranges by global degree rank (round-robin);
    core c owns its dsts and all edges pointing into them.  Own dsts are
    degree-sorted and tiled 128 at a time.
  - Layer 1 does NO gathering at all: the host pre-permutes x into per-core
    edge-slot order (xpair[:, (t,k,p)] = x[src of slot k of dst p in tile t]),
    so z1aug = xpair-block @ [W1 | W1@AL] lands in PSUM already in the
    [128 dsts, slot] layout the edge-softmax needs.  Pad slots use a host
    vector v with v@(W1@AL) = -100 so exp(lrelu(el+er)) vanishes.
  - Edge softmax: e = exp(lrelu(el_src + er_dst)), denominators via ACT
    accum, weighting + segment-sum via DVE binary-tree folds over slots.
  - Layer-2 projection z2aug = h @ [W2|W2@AL2|W2@AR2] from the layer-1 tiles
    (PE transpose), stored bf16 into a per-core shard in tile/perm order,
    AllGather -> tab2 (Shared), copied to a core-private table, then layer 2
    runs the dma_gather machinery: 256B bf16 rows [z2|el2,er2 f32], int16
    indices via two overlapping windows A=[0,32768) / B=[ROWS-32768,ROWS).
  - dma_gather throughput notes: descriptor GENERATION on the Q7 cluster is
    the floor (~4.2us per 1024-idx instruction, serial across the cluster);
    the drain is spread over 4 SWDGE queues (idx data must be replicated
    into partitions [32q,32q+32) for queue q -- see _pack_idx).  >1024 idxs
    per instruction overflows the SWDGE ring.
"""
import sys

sys.path.insert(0, "/opt/trn_rl_repo")

import numpy as np

import concourse.bass as bass
import concourse.mybir as mybir
import concourse.tile as tile_mod
from concourse import library_config
from concourse.library_overlay import lower_extended_insts
from concourse.tile import TileContext
from concourse.bass_utils import run_bass_kernel_spmd

F32 = mybir.dt.float32
BF16 = mybir.dt.bfloat16
I16 = mybir.dt.int16
AF = mybir.ActivationFunctionType
ALU = mybir.AluOpType

NEG_SLOPE = 0.2
SENT_EL = -1.0e30
L1C = 264          # g-slot cols (bf16): z(256) + el(4 f32 bitcast = 8)
L2C = 128          # tab2 row cols (bf16): z2(64) + el2,er2(f32 bitcast = 4)


# ---------------------------------------------------------------------------
# Workaround: this walrus build rejects Drain instructions with >1 sync wait.
def _patched_drain_and_barrier(self, tick_clock, wait_clock):
    nc = self.nc
    probe = nc.sync.drain()
    wait_clock.add_sem_waits(
        probe.ins, tile_mod.ScopedClock({None: tick_clock.global_clock})
    )
    si = probe.ins.sync_info
    waits = list(si.on_wait) if si is not None else []
    if len(waits) > 1:
        bb = nc.cur_bb.bb
        popped = bb.instructions.pop()
        assert popped is probe.ins
        by_name = {}
        for h in self.sems.allocated().values():
            by_name[h.name] = h
        for w in waits:
            assert w.wait_mode == "sem-ge-imm", w
            nc.sync.wait_ge(by_name[w.ant_name], w.wait_value)
        nc.sync.drain()
    nc.all_engine_barrier()
    popped_p = nc._tile_sem_poison_stack.pop()
    assert popped_p is self._sem_poison
    nc.clear_and_free_semaphores(list(self.sems.allocated().values()))
    nc.all_engine_barrier()


TileContext._drain_and_barrier = _patched_drain_and_barrier

_wsplit_n = 0


def _split_multi_waits(nc, keep=1):
    """This walrus build allows at most one sync-wait per instruction; hoist
    extra waits onto dedicated EventSemaphore instructions just before."""
    global _wsplit_n
    for f in nc.m.functions:
        for bb in f.blocks:
            need = any(
                inst.sync_info is not None and len(inst.sync_info.on_wait) > keep
                for inst in bb.instructions
            )
            if not need:
                continue
            newlist = []
            for inst in bb.instructions:
                si = inst.sync_info
                if si is not None and len(si.on_wait) > keep:
                    waits = list(si.on_wait)
                    for w in waits[:-keep]:
                        ev = mybir.InstEventSemaphore(
                            name=f"WSPLIT-{_wsplit_n}", ins=[], outs=[])
                        _wsplit_n += 1
                        ev.engine = inst.engine
                        ev.sync_info = mybir.SyncInfo(on_wait=[w], on_update=[])
                        newlist.append(ev)
                    inst.sync_info = mybir.SyncInfo(
                        on_wait=waits[-keep:], on_update=list(si.on_update))
                newlist.append(inst)
            try:
                bb.instructions[:] = newlist
            except TypeError:
                while len(bb.instructions):
                    bb.instructions.pop()
                for inst in newlist:
                    bb.instructions.append(inst)


# ---------------------------------------------------------------------------
def _pack_idx(logical):
    """int16 idx list -> [128, n/16] wrapped/replicated layout for dma_gather.

    Queue q's Q7 cpu pair reads idx data through SIMD channels
    [32q, 32q+32) -> SBUF partitions 32q..32q+31 of the idx tile, so the
    16-partition wrap is replicated 8x to cover all four queues."""
    n = len(logical)
    assert n % 16 == 0
    a = np.asarray(logical, np.int16).reshape(n // 16, 16).T
    out = np.empty((128, n // 16), np.int16)
    for b in range(8):
        out[16 * b:16 * (b + 1)] = a
    return out


class Cfg:
    def __init__(self, N, E, lim=32768):
        self.N = N
        self.E = E
        self.NC = 8
        self.IN = 256
        self.HID = 64
        self.H1 = 4
        self.OUT = 64
        self.OWN = N // self.NC
        self.OWNP = -(-self.OWN // 128) * 128
        self.ROWS = self.NC * self.OWNP
        self.LIM = lim                      # rows addressable by one window
        self.TBOFF = max(self.ROWS - lim, 0)  # start row of window B
        self.NT = self.OWNP // 128          # dst tiles per core
        self.SENT_A = self.OWN              # sentinel row (block 0 pad row)
        self.SENT_B = (self.NC - 1) * self.OWNP + self.OWN
        assert self.SENT_B < self.ROWS
        assert self.SENT_B - self.TBOFF < lim


def prep(cfg, x, W1, al1, ar1, b1, W2, al2, ar2, b2, src, dst):
    """Host-side graph partitioning / staging.  Returns (in_maps, sched, post)."""
    N, E, NC = cfg.N, cfg.E, cfg.NC
    IN, HID, H1, OUT = cfg.IN, cfg.HID, cfg.H1, cfg.OUT

    x = np.asarray(x, np.float32)
    src = np.asarray(src)
    dst = np.asarray(dst)
    W1 = np.asarray(W1, np.float32)
    W2 = np.asarray(W2, np.float32)
    al1 = np.asarray(al1, np.float32)
    ar1 = np.asarray(ar1, np.float32)
    al2 = np.asarray(al2, np.float32)
    ar2 = np.asarray(ar2, np.float32)
    b1 = np.asarray(b1, np.float32)
    b2 = np.asarray(b2, np.float32)

    # parameter transforms
    AL1 = np.zeros((H1 * HID, H1), np.float32)
    AR1 = np.zeros((H1 * HID, H1), np.float32)
    for h in range(H1):
        AL1[h * HID:(h + 1) * HID, h] = al1[h]
        AR1[h * HID:(h + 1) * HID, h] = ar1[h]
    W1aug = np.concatenate([W1, W1 @ AL1], axis=1)          # [IN, 260]
    W1ar = W1 @ AR1                                         # [IN, 4]
    AL2 = al2.reshape(OUT, 1)
    AR2 = ar2.reshape(OUT, 1)
    W2aug = np.concatenate([W2, W2 @ AL2, W2 @ AR2], axis=1)  # [256, 66]

    # pad-slot column: v @ (W1@AL1) = -100 (min-norm), so pad edges score
    # exp(lrelu(-100 + er)) ~ 0 while z_pad stays bounded.
    W1AL1 = W1 @ AL1
    v = np.linalg.lstsq(W1AL1.T, np.full(H1, -100.0, np.float32),
                        rcond=None)[0].astype(np.float32)

    bf = mybir.dt.np(BF16)
    xb = x.astype(bf)
    vb = v.astype(bf)
    el_pad = vb.astype(np.float32) @ W1AL1
    assert el_pad.max() < -50.0, el_pad

    # ownership: round-robin by global degree rank -> per-tile max degrees
    # align across cores (shared program, minimal padding)
    deg_g = np.bincount(dst, minlength=N)
    grank = np.argsort(-deg_g, kind="stable")
    owner = np.empty(N, np.int64)
    local_rank = np.empty(N, np.int64)
    owner[grank] = np.arange(N) % NC
    local_rank[grank] = np.arange(N) // NC
    nodes_by_core = [grank[c::NC] for c in range(NC)]   # local-rank order

    edge_by_core = [np.nonzero(owner[dst] == c)[0] for c in range(NC)]

    # per-core degree sort (tile order = perm order; shard is laid out in
    # perm order so tile outputs land contiguously -- no scatter needed).
    # Two passes: pass-0 perms from degree only -> provisional rows; pass-1
    # refines the tie-break with per-dst must-A counts under those rows.
    def _perms_to_rows(perms_):
        rows = np.empty(N, np.int64)
        for c in range(NC):
            inv = np.empty(cfg.OWN, np.int64)
            inv[perms_[c][:cfg.OWN]] = np.arange(cfg.OWN)
            rows[nodes_by_core[c]] = c * cfg.OWNP + inv
        return rows

    deg_by_core = []
    for c in range(NC):
        eid = edge_by_core[c]
        dloc = local_rank[dst[eid]]
        deg_by_core.append(np.bincount(dloc, minlength=cfg.OWN))

    def _mk_perms(rows_prov):
        perms_ = []
        for c in range(NC):
            eid = edge_by_core[c]
            dloc = local_rank[dst[eid]]
            if rows_prov is None:
                order = np.argsort(-deg_by_core[c], kind="stable")
            else:
                mA = np.bincount(dloc[rows_prov[src[eid]] < cfg.TBOFF],
                                 minlength=cfg.OWN)
                order = np.lexsort((mA, deg_by_core[c]))[::-1]
            perms_.append(np.concatenate(
                [order, np.full(cfg.OWNP - cfg.OWN, order[-1], np.int64)]))
        return perms_

    perms = _mk_perms(_perms_to_rows(_mk_perms(None)))

    # table row of node n = owner*OWNP + position of n in its core's perm
    row_of_node = _perms_to_rows(perms)
    rows_src = row_of_node[src]

    # per-core (src-rows, src-nodes) bucketed by local dst
    per_core_bydst = []
    for c in range(NC):
        eid = edge_by_core[c]
        dloc = local_rank[dst[eid]]
        so = np.argsort(dloc, kind="stable")
        srows_sorted = rows_src[eid][so]
        snode_sorted = src[eid][so]
        starts = np.searchsorted(dloc[so], np.arange(cfg.OWN + 1))
        per_core_bydst.append(
            [(srows_sorted[starts[i]:starts[i + 1]],
              snode_sorted[starts[i]:starts[i + 1]]) for i in range(cfg.OWN)])

    # ---- G1 schedule: per-tile slot count = max degree across cores ----
    K1s = []
    for t in range(cfg.NT):
        m = 1
        for c in range(NC):
            dts = perms[c][t * 128:(t + 1) * 128]
            hi = min((t + 1) * 128, cfg.OWN)
            if t * 128 < cfg.OWN:
                m = max(m, int(deg_by_core[c][dts[:hi - t * 128]].max()))
        K1s.append(m)
    E1 = sum(K1s) * 128

    # ---- G2 schedule: joint per-tile (alpha, beta) A/B window feasibility ----
    tiles_ab = []
    for c in range(NC):
        bydst = per_core_bydst[c]
        perm = perms[c]
        stats = []
        for t in range(cfg.NT):
            dts = perm[t * 128:(t + 1) * 128]
            mA = np.zeros(128, np.int64)
            mB = np.zeros(128, np.int64)
            dg = np.zeros(128, np.int64)
            for i in range(128):
                if t * 128 + i >= cfg.OWN:
                    continue
                rs = bydst[dts[i]][0]
                dg[i] = len(rs)
                mA[i] = int((rs < cfg.TBOFF).sum())
                mB[i] = int((rs >= cfg.LIM).sum())
            stats.append((mA, mB, dg))
        tiles_ab.append(stats)

    Ks = []
    for t in range(cfg.NT):
        alo = max(int(tiles_ab[c][t][0].max()) for c in range(NC))
        ahi = max(int(tiles_ab[c][t][2].max()) for c in range(NC))
        best = None
        for alpha in range(alo, ahi + 1):
            beta = 0
            for c in range(NC):
                mA, mB, dg = tiles_ab[c][t]
                beta = max(beta, int(np.maximum(
                    mB, dg - np.minimum(alpha, dg - mB)).max()))
            if best is None or alpha + beta < best[0] + best[1]:
                best = (alpha, beta)
        Ks.append((max(best[0], 1), max(best[1], 1)))

    # ---- per-core staging ----
    idx_all, xpairs, xTps = [], [], []
    for c in range(NC):
        perm = perms[c]
        bydst = per_core_bydst[c]
        # xpair: edge-slot-ordered x columns
        srcflat = np.full(E1, -1, np.int64)
        base = 0
        for t in range(cfg.NT):
            K = K1s[t]
            dts = perm[t * 128:(t + 1) * 128]
            for i in range(128):
                if t * 128 + i >= cfg.OWN:
                    continue
                nodes = bydst[dts[i]][1]
                srcflat[base + np.arange(len(nodes)) * 128 + i] = nodes
            base += K * 128
        xp = np.empty((E1, IN), bf)
        m = srcflat >= 0
        xp[m] = xb[srcflat[m]]
        xp[~m] = vb
        xpairs.append(np.ascontiguousarray(xp.T))

        # G2 idx tables (A/B windows)
        cols = []
        for t in range(cfg.NT):
            a_t, b_t = Ks[t]
            Aidx = np.full((a_t, 128), cfg.SENT_A, np.int64)
            Bidx = np.full((b_t, 128), cfg.SENT_B - cfg.TBOFF, np.int64)
            dts = perm[t * 128:(t + 1) * 128]
            for i in range(128):
                if t * 128 + i >= cfg.OWN:
                    continue
                rs = bydst[dts[i]][0]
                isA = rs < cfg.TBOFF
                isB = rs >= cfg.LIM
                flex = rs[~isA & ~isB]
                Alist = list(rs[isA])
                Blist = list(rs[isB])
                nA = min(a_t, len(Alist) + len(flex))
                take = nA - len(Alist)
                Alist += list(flex[:take])
                Blist += list(flex[take:])
                assert len(Alist) <= a_t and len(Blist) <= b_t, (t, i)
                Aidx[:len(Alist), i] = Alist
                Bidx[:len(Blist), i] = np.asarray(Blist) - cfg.TBOFF
            assert Aidx.max() < cfg.LIM
            cols.append(_pack_idx(Aidx.reshape(-1)))
            cols.append(_pack_idx(Bidx.reshape(-1)))
        idx_all.append(np.concatenate(cols, axis=1))
        xTps.append(np.ascontiguousarray(
            x[nodes_by_core[c][perm]].astype(bf).T))

    sent2 = np.zeros((1, L2C), bf)
    sent2.view(np.uint16)[0, 64:68] = np.array(
        [SENT_EL, SENT_EL], np.float32).view(np.uint16)

    b1bc = np.broadcast_to(b1.reshape(1, -1), (128, H1 * HID)).astype(bf)
    b2bc = np.broadcast_to(b2.reshape(1, -1), (128, OUT)).copy()
    ident = np.eye(128, dtype=bf)

    in_maps = []
    for c in range(NC):
        in_maps.append(
            {
                "xpair": xpairs[c],
                "xTp": xTps[c],
                "W1aug": W1aug.astype(bf),
                "W1ar": W1ar.astype(bf),
                "W2aug": W2aug.astype(bf),
                "b1bc": b1bc,
                "b2bc": b2bc,
                "ident": ident,
                "sent2": sent2,
                "idx_all": idx_all[c],
            }
        )
    sched = {"Ks": Ks, "K1s": K1s, "E1": E1, "idx_cols": idx_all[0].shape[1]}
    post = {"perms": perms, "nodes_by_core": nodes_by_core}
    return in_maps, sched, post


# ---------------------------------------------------------------------------
def build(cfg, sched, phases=4, g1_mode=5, g2_mode=1, reps=1, nq=4, chunk=8,
          g1b=4, g2b=6, zpb=2, tpb=1, xeb=3, shared_gather=False):
    Ks = sched["Ks"]
    K1s = sched["K1s"]
    E1 = sched["E1"]
    nc = bass.Bass(num_swdge_queues=nq)
    _qctr = [0]

    def nxq():
        q = _qctr[0] % nq
        _qctr[0] += 1
        return q
    IN, H1, HID, OUT = cfg.IN, cfg.H1, cfg.HID, cfg.OUT

    def P(name, shape, dt=F32):
        return nc.declare_dram_parameter(name, list(shape), dt, isOutput=False)

    xpair = P("xpair", [IN, E1], BF16)
    xTp = P("xTp", [IN, cfg.OWNP], BF16)
    W1a = P("W1aug", [IN, 260], BF16)
    W1r = P("W1ar", [IN, 4], BF16)
    W2a = P("W2aug", [IN, 66], BF16)
    b1b = P("b1bc", [128, 256], BF16)
    b2b = P("b2bc", [128, OUT])
    idn = P("ident", [128, 128], BF16)
    sent2 = P("sent2", [1, L2C], BF16)
    idx_all = P("idx_all", [128, sched["idx_cols"]], I16)
    outp = nc.declare_dram_parameter("outperm", [cfg.OWNP, OUT], F32, isOutput=True)

    shard = nc.dram_tensor("shard", [cfg.OWNP, L2C], BF16)
    tab2s = nc.dram_tensor("tab2s", [cfg.ROWS, L2C], BF16, addr_space="Shared")
    if shared_gather:
        tab2 = tab2s
    else:
        tab2 = nc.dram_tensor("tab2", [cfg.ROWS, L2C], BF16)

    _regs = {}

    def nreg(v):
        if v not in _regs:
            _regs[v] = nc.gpsimd.to_reg(v)
        return _regs[v]

    with TileContext(nc) as tc:
        nc.gpsimd.load_library(library_config.mlp)
        with tc.tile_pool(name="const", bufs=1) as cp:
            w1a = cp.tile([128, 2 * 260], BF16, tag="w1a")
            w1r = cp.tile([128, 2 * 4], BF16, tag="w1r")
            w2a = cp.tile([128, 2 * 66], BF16, tag="w2a")
            b1s = cp.tile([128, 256], BF16, tag="b1s")
            b2s = cp.tile([128, OUT], F32, tag="b2s")
            ids = cp.tile([128, 128], BF16, tag="ids")
            er1 = cp.tile([128, cfg.NT * 4], F32, tag="er1")
            er2 = cp.tile([128, cfg.NT], F32, tag="er2")
            itall = cp.tile([128, sched["idx_cols"]], I16, tag="itall")
            nc.sync.dma_start(out=itall[:], in_=idx_all[:])
            for k in range(2):
                nc.sync.dma_start(out=w1a[:, k * 260:(k + 1) * 260],
                                  in_=W1a[k * 128:(k + 1) * 128, :])
                nc.sync.dma_start(out=w1r[:, k * 4:(k + 1) * 4],
                                  in_=W1r[k * 128:(k + 1) * 128, :])
                nc.sync.dma_start(out=w2a[:, k * 66:(k + 1) * 66],
                                  in_=W2a[k * 128:(k + 1) * 128, :])
            nc.sync.dma_start(out=b1s[:], in_=b1b[:])
            nc.sync.dma_start(out=b2s[:], in_=b2b[:])
            nc.sync.dma_start(out=ids[:], in_=idn[:])

            for _rep in range(reps):
                # ---------------- er1 mini-pass (own pool region) ----------
                ctxG1 = nc.named_scope("phaseG1"); ctxG1.__enter__()
                ESUP = 8
                with tc.tile_pool(name="ex1", bufs=2) as exp_, \
                     tc.tile_pool(name="ep1", bufs=2, space="PSUM") as epp:
                    for st in range(-(-cfg.NT // ESUP) if phases >= 1 else 0):
                        t0 = st * ESUP
                        ntl = min(ESUP, cfg.NT - t0)
                        cols = ntl * 128
                        xb = exp_.tile([128, 2, cols], BF16, tag="xb1")
                        for k in range(2):
                            nc.sync.dma_start(
                                out=xb[:, k, :],
                                in_=xTp[k * 128:(k + 1) * 128,
                                        t0 * 128:t0 * 128 + cols])
                        for i in range(ntl):
                            ep = epp.tile([128, 4], F32, tag="ep")
                            for k in range(2):
                                nc.tensor.matmul(
                                    ep[:], xb[:, k, i * 128:(i + 1) * 128],
                                    w1r[:, k * 4:(k + 1) * 4],
                                    start=(k == 0), stop=(k == 1))
                            nc.vector.tensor_copy(
                                er1[:, (t0 + i) * 4:(t0 + i) * 4 + 4], ep[:])

                # ---------------- phase G1: matmul-direct layer 1 ----------
                # 2-way software interleave: stages of tile pairs are emitted
                # alternately so each in-order engine queue always has an
                # independent chain behind a stalled wait.
                GRP = 3           # psum slots per group (bank-aligned slots)
                ecoffs = np.concatenate([[0], np.cumsum(np.asarray(K1s) * 128)])
                with tc.tile_pool(name="xe", bufs=xeb) as xep, \
                     tc.tile_pool(name="g1", bufs=g1b) as gp, \
                     tc.tile_pool(name="w1p", bufs=3) as wp, \
                     tc.tile_pool(name="hb", bufs=3) as hp, \
                     tc.tile_pool(name="s2", bufs=3) as s2p, \
                     tc.tile_pool(name="sm", bufs=4) as smp, \
                     tc.tile_pool(name="zp", bufs=zpb, space="PSUM") as zpp, \
                     tc.tile_pool(name="tp", bufs=tpb, space="PSUM") as tpp:

                    def g1_s1(t):
                        K = K1s[t]
                        ecoff = int(ecoffs[t])
                        xe = xep.tile([128, 2, K * 128], BF16, tag="xe")
                        for k in range(2):
                            nc.sync.dma_start(
                                out=xe[:, k, :],
                                in_=xpair[k * 128:(k + 1) * 128,
                                          ecoff:ecoff + K * 128])
                        g = gp.tile([128, K * L1C], BF16, tag="g")
                        ga = g[:]
                        gf = ga.bitcast(F32)
                        for g0 in range(0, K, GRP):
                            n = min(GRP, K - g0)
                            zpg = zpp.tile([128, GRP * 512], F32, tag="zpg")
                            for j in range(n):
                                for kb in range(2):
                                    nc.tensor.matmul(
                                        zpg[:, j * 512:j * 512 + 260],
                                        xe[:, kb, (g0 + j) * 128:(g0 + j + 1) * 128],
                                        w1a[:, kb * 260:(kb + 1) * 260],
                                        start=(kb == 0), stop=(kb == 1))
                            zpa = zpg[:]
                            # z cols: one strided ACT copy per group
                            nc.scalar.copy(
                                bass.AP(ga.tensor, ga.offset + g0 * L1C,
                                        [ga.ap[0], [L1C, n], [1, 256]]),
                                bass.AP(zpa.tensor, zpa.offset,
                                        [zpa.ap[0], [512, n], [1, 256]]))
                            # el cols: fused + er1 (one DVE add per group)
                            era = er1[:]
                            nc.vector.tensor_tensor(
                                bass.AP(gf.tensor,
                                        gf.offset + g0 * (L1C // 2) + 128,
                                        [gf.ap[0], [L1C // 2, n], [1, 4]]),
                                bass.AP(zpa.tensor, zpa.offset + 256,
                                        [zpa.ap[0], [512, n], [1, 4]]),
                                bass.AP(era.tensor, era.offset + 4 * t,
                                        [era.ap[0], [0, n], [1, 4]]),
                                op=ALU.add)
                        return dict(t=t, K=K, g=g, ga=ga, gf=gf)

                    def g1_s2(st):
                        if g1_mode < 1:
                            return
                        K, ga, gf = st["K"], st["ga"], st["gf"]
                        w = wp.tile([128, K * 4], BF16, tag="w")
                        s = smp.tile([128, 4], F32, tag="s")
                        rs = smp.tile([128, 4], F32, tag="rs")
                        # w laid out h-major: w[:, h*K + k]
                        el_all = bass.AP(gf.tensor, gf.offset + 128,
                                         [gf.ap[0], [L1C // 2, K], [1, 4]])
                        wa = w[:]
                        w_hm = bass.AP(wa.tensor, wa.offset,
                                       [wa.ap[0], [1, K], [K, 4]])
                        nc.scalar.activation(w_hm, el_all, AF.Prelu,
                                             scale=1.0, alpha=NEG_SLOPE)
                        for h in range(4):
                            nc.scalar.activation(
                                w[:, h * K:(h + 1) * K],
                                w[:, h * K:(h + 1) * K], AF.Exp,
                                accum_out=s[:, h:h + 1])
                        nc.vector.tensor_scalar_max(s[:], s[:], 1e-30)
                        nc.vector.reciprocal(rs[:], s[:])
                        st["w"], st["rs"] = w, rs

                    def g1_s3(st):
                        if g1_mode < 2:
                            return
                        K, ga = st["K"], st["ga"]
                        wa = st["w"][:]
                        gz = bass.AP(ga.tensor, ga.offset,
                                     [ga.ap[0], [L1C, K], [HID, 4], [1, HID]])
                        wbc = bass.AP(wa.tensor, wa.offset,
                                      [wa.ap[0], [1, K], [K, 4], [0, HID]])
                        nc.vector.tensor_tensor(gz, gz, wbc, op=ALU.mult)
                        Kc = K
                        while Kc > 1 and g1_mode >= 3:
                            half = Kc // 2
                            m = Kc - half
                            lo = bass.AP(ga.tensor, ga.offset,
                                         [ga.ap[0], [L1C, half], [1, 256]])
                            hi = bass.AP(ga.tensor, ga.offset + m * L1C,
                                         [ga.ap[0], [L1C, half], [1, 256]])
                            nc.vector.tensor_tensor(lo, lo, hi, op=ALU.add)
                            Kc = m

                    def g1_s4(st):
                        if g1_mode < 4:
                            return
                        t, ga, rs = st["t"], st["ga"], st["rs"]
                        hb = hp.tile([128, 256], BF16, tag="hb")
                        acc = bass.AP(ga.tensor, ga.offset,
                                      [ga.ap[0], [HID, 4], [1, HID]])
                        hba = hb[:]
                        hb4 = bass.AP(hba.tensor, hba.offset,
                                      [hba.ap[0], [HID, 4], [1, HID]])
                        rsa = rs[:]
                        rsb = bass.AP(rsa.tensor, rsa.offset,
                                      [rsa.ap[0], [1, 4], [0, HID]])
                        nc.vector.tensor_tensor(hb4, acc, rsb, op=ALU.mult)
                        nc.vector.tensor_tensor(hb[:], hb[:], b1s[:], op=ALU.add)
                        tmp = hp.tile([128, 256], BF16, tag="elutmp")
                        nc.vector.tensor_scalar_min(tmp[:], hb[:], 0.0)
                        nc.vector.tensor_scalar_max(hb[:], hb[:], 0.0)
                        nc.scalar.activation(tmp[:], tmp[:], AF.Exp)
                        nc.vector.tensor_tensor(hb[:], hb[:], tmp[:], op=ALU.add)
                        nc.vector.tensor_scalar_add(hb[:], hb[:], -1.0)
                        if g1_mode < 5:
                            return
                        zp2 = tpp.tile([128, 66], F32, tag="z2p")
                        for k in range(2):
                            tp = tpp.tile([128, 128], BF16, tag="tp")
                            nc.tensor.transpose(tp[:], hb[:, k * 128:(k + 1) * 128],
                                                ids[:])
                            hT = s2p.tile([128, 128], BF16, tag="hT")
                            nc.scalar.copy(hT[:], tp[:])
                            nc.tensor.matmul(zp2[:], hT[:],
                                             w2a[:, k * 66:(k + 1) * 66],
                                             start=(k == 0), stop=(k == 1))
                        z2sb = s2p.tile([128, L2C], BF16, tag="z2sb")
                        nc.vector.tensor_copy(er2[:, t:t + 1], zp2[:, 65:66])
                        nc.scalar.copy(z2sb[:, 0:64], zp2[:, 0:64])
                        nc.vector.tensor_copy(
                            z2sb.bitcast(F32)[:, 32:34], zp2[:, 64:66])
                        nc.scalar.dma_start(
                            out=shard[t * 128:(t + 1) * 128, :], in_=z2sb[:])

                    NT1 = cfg.NT if phases >= 1 else 0
                    for i in range(0, NT1, 2):
                        ts = [t for t in (i, i + 1) if t < NT1]
                        sts = [g1_s1(t) for t in ts]
                        for st in sts:
                            g1_s2(st)
                        for st in sts:
                            g1_s3(st)
                        for st in sts:
                            g1_s4(st)
                    # pad-slot sentinel row (overwrites the last tile's write)
                    if phases >= 2:
                        nc.scalar.dma_start(
                            out=shard[cfg.OWN:cfg.OWN + 1, :], in_=sent2[:])

                # ---------------- allgather (+ local shadow copy) ----------
                ctxG1.__exit__(None, None, None)
                if phases >= 3:
                    with nc.named_scope("phaseAG"):
                        nc.gpsimd.collective_compute(
                            "AllGather", ALU.bypass, ins=[shard[:]],
                            outs=[tab2s[:]],
                            replica_groups=[list(range(cfg.NC))])
                        if not shared_gather:
                            half = cfg.ROWS // 2
                            nc.sync.dma_start(out=tab2[0:half, :],
                                              in_=tab2s[0:half, :])
                            nc.scalar.dma_start(out=tab2[half:, :],
                                                in_=tab2s[half:, :])

                # ---------------- phase G2: layer 2 ----------------
                ioff = 0
                Kmax = max(a + b for a, b in Ks)
                ctxG2 = nc.named_scope("phaseG2"); ctxG2.__enter__()
                with tc.tile_pool(name="g2", bufs=g2b) as gp2, \
                     tc.tile_pool(name="gf2", bufs=2) as gfp, \
                     tc.tile_pool(name="w2p", bufs=2) as wp2, \
                     tc.tile_pool(name="ob", bufs=2) as op_, \
                     tc.tile_pool(name="sm2", bufs=4) as smp2:
                    for t in range(cfg.NT if phases >= 4 else 0):
                        a_t, b_t = Ks[t]
                        K = a_t + b_t
                        icols = K * 8
                        it = itall[:, ioff:ioff + icols]
                        g = gp2.tile([128, Kmax * L2C], BF16, tag="g2")
                        gv = g[:, :K * L2C].rearrange("p (k c) -> p k c", c=L2C)
                        for c0 in range(0, a_t, chunk):
                            n = min(chunk, a_t - c0)
                            nc.gpsimd.dma_gather(
                                out_ap=gv[:, c0:c0 + n, :], in_ap=tab2[:],
                                idxs_ap=it[:, c0 * 8:(c0 + n) * 8],
                                num_idxs=128 * n, queue_num=nxq(),
                                num_idxs_reg=nreg(128 * n), elem_size=L2C)
                        for c0 in range(0, b_t, chunk):
                            n = min(chunk, b_t - c0)
                            nc.gpsimd.dma_gather(
                                out_ap=gv[:, a_t + c0:a_t + c0 + n, :],
                                in_ap=tab2[cfg.TBOFF:, :],
                                idxs_ap=it[:, (a_t + c0) * 8:(a_t + c0 + n) * 8],
                                num_idxs=128 * n, queue_num=nxq(),
                                num_idxs_reg=nreg(128 * n), elem_size=L2C)
                        if g2_mode < 1:
                            ioff += icols
                            continue
                        w2t = wp2.tile([128, Kmax], F32, tag="w2t")
                        s2 = smp2.tile([128, 1], F32, tag="s2")
                        rs2 = smp2.tile([128, 1], F32, tag="rs2")
                        ga = g[:]
                        gaf = ga.bitcast(F32)
                        el2 = bass.AP(gaf.tensor, gaf.offset + 32,
                                      [gaf.ap[0], [L2C // 2, K]])
                        nc.scalar.activation(
                            w2t[:, :K], el2, AF.Prelu,
                            bias=er2[:, t:t + 1], scale=1.0, alpha=NEG_SLOPE)
                        nc.scalar.activation(
                            w2t[:, :K], w2t[:, :K], AF.Exp, accum_out=s2[:])
                        nc.vector.tensor_scalar_max(s2[:], s2[:], 1e-30)
                        nc.vector.reciprocal(rs2[:], s2[:])
                        wa = w2t[:]
                        gf2 = gfp.tile([128, Kmax * OUT], F32, tag="gf2")
                        g2fa = gf2[:]
                        gz = bass.AP(ga.tensor, ga.offset,
                                     [ga.ap[0], [L2C, K], [1, OUT]])
                        gzf = bass.AP(g2fa.tensor, g2fa.offset,
                                      [g2fa.ap[0], [OUT, K], [1, OUT]])
                        wbc = bass.AP(wa.tensor, wa.offset,
                                      [wa.ap[0], [1, K], [0, OUT]])
                        nc.vector.tensor_tensor(gzf, gz, wbc, op=ALU.mult)
                        Kc = K
                        while Kc > 1:
                            half = Kc // 2
                            m = Kc - half
                            lo = bass.AP(g2fa.tensor, g2fa.offset,
                                         [g2fa.ap[0], [OUT, half], [1, OUT]])
                            hi = bass.AP(g2fa.tensor, g2fa.offset + m * OUT,
                                         [g2fa.ap[0], [OUT, half], [1, OUT]])
                            nc.vector.tensor_tensor(lo, lo, hi, op=ALU.add)
                            Kc = m
                        ob = op_.tile([128, OUT], F32, tag="ob")
                        nc.vector.tensor_scalar_mul(ob[:], gf2[:, 0:OUT], rs2[:])
                        nc.vector.tensor_tensor(ob[:], ob[:], b2s[:], op=ALU.add)
                        nc.sync.dma_start(
                            out=outp[t * 128:(t + 1) * 128, :], in_=ob[:])
                        ioff += icols
                ctxG2.__exit__(None, None, None)

    _split_multi_waits(nc)
    lower_extended_insts(nc)
    return nc


# ---------------------------------------------------------------------------
_memo = {}


def run(cfg, inputs, trace=False, **bkw):
    in_maps, sched, post = prep(cfg, **inputs)
    key = (cfg.N, cfg.E, cfg.LIM, tuple(sched["Ks"]), tuple(sched["K1s"]),
           tuple(sorted(bkw.items())))
    if key not in _memo:
        _memo[key] = build(cfg, sched, **bkw)
    nc = _memo[key]
    res = run_bass_kernel_spmd(
        nc, in_maps, list(range(cfg.NC)), trace=trace)
    out = np.zeros((cfg.N, cfg.OUT), np.float32)
    for c in range(cfg.NC):
        op = res.results[c]["outperm"]
        perm = post["perms"][c]
        out[post["nodes_by_core"][c][perm[:cfg.OWN]]] = op[:cfg.OWN]
    return out, res


def _make_exec(cfg, nc, in_maps):
    """Compile nc into a donated sharded callable; returns (call, decode)."""
    import jax
    from jax.experimental.shard_map import shard_map
    from jax.sharding import Mesh, PartitionSpec

    from concourse import bass2jax

    bass2jax.install_neuronx_cc_hook()
    partition_name = nc.partition_id_tensor.name if nc.partition_id_tensor else None
    in_names, out_names, out_avals, zero_outs = [], [], [], []
    for alloc in nc.m.functions[0].allocations:
        if not isinstance(alloc, mybir.MemoryLocationSet):
            continue
        name = alloc.memorylocations[0].name
        if alloc.kind == "ExternalInput":
            if name != partition_name:
                in_names.append(name)
        elif alloc.kind == "ExternalOutput":
            out_names.append(name)
            shape = tuple(alloc.tensor_shape)
            dtype = mybir.dt.np(alloc.dtype)
            out_avals.append(jax.core.ShapedArray(shape, dtype))
            zero_outs.append(np.zeros(shape, dtype))
    n_params = len(in_names)
    n_outs = len(out_avals)
    all_in_names = list(in_names) + list(out_names)
    if partition_name is not None:
        all_in_names.append(partition_name)
    donate = tuple(range(n_params, n_params + n_outs))

    def _body(*args):
        operands = list(args)
        if partition_name is not None:
            operands.append(bass2jax.partition_id_tensor())
        outs = bass2jax._bass_exec_p.bind(
            *operands,
            out_avals=tuple(out_avals),
            in_names=tuple(all_in_names),
            out_names=tuple(out_names),
            lowering_input_output_aliases=(),
            sim_require_finite=True,
            sim_require_nnan=True,
            nc=nc,
        )
        return tuple(outs)

    NCOR = cfg.NC
    devices = jax.devices()[:NCOR]
    mesh = Mesh(np.asarray(devices), ("core",))
    in_specs = (PartitionSpec("core"),) * (n_params + n_outs)
    out_specs = (PartitionSpec("core"),) * len(out_names)
    sharded = jax.jit(
        shard_map(_body, mesh=mesh, in_specs=in_specs, out_specs=out_specs,
                  check_rep=False),
        donate_argnums=donate, keep_unused=True)
    sharding = jax.sharding.NamedSharding(mesh, PartitionSpec("core"))
    concat_in = [
        jax.device_put(
            np.concatenate([np.asarray(in_maps[c][n]) for c in range(NCOR)],
                           axis=0), sharding)
        for n in in_names
    ]

    def fresh_zeros():
        return [
            jax.device_put(
                np.zeros((NCOR * z.shape[0], *z.shape[1:]), z.dtype), sharding)
            for z in zero_outs
        ]

    def call(timed=False):
        import time
        zs = fresh_zeros()
        jax.block_until_ready(zs)
        t0 = time.perf_counter()
        o = sharded(*concat_in, *zs)
        jax.block_until_ready(o)
        return (time.perf_counter() - t0) if timed else o

    def decode(out_arrs):
        return [
            {n: np.asarray(out_arrs[i]).reshape(NCOR, *out_avals[i].shape)[c]
             for i, n in enumerate(out_names)}
            for c in range(NCOR)
        ]

    return call, decode


def _nc_for(cfg, sched, reps, **bkw):
    key = (cfg.N, cfg.E, cfg.LIM, tuple(sched["Ks"]), tuple(sched["K1s"]),
           reps, tuple(sorted(bkw.items())))
    if key not in _memo:
        _memo[key] = build(cfg, sched, reps=reps, **bkw)
    return _memo[key]


def run_bench_pair(cfg, inputs, iters=10, reps_lo=1, reps_hi=11, **bkw):
    """Correctness output plus interleaved wall-time samples of a reps_lo and
    a reps_hi NEFF.  Interleaving makes both variants see the same host/
    tunnel conditions, so min(hi)-min(lo) differences out the fixed dispatch
    overhead and isolates hardware execution time."""
    prepped = prep(cfg, **inputs)
    in_maps, sched, post = prepped
    call_lo, decode = _make_exec(cfg, _nc_for(cfg, sched, reps_lo, **bkw), in_maps)
    call_hi, _ = _make_exec(cfg, _nc_for(cfg, sched, reps_hi, **bkw), in_maps)
    results = decode(call_lo())
    call_hi()  # warm the hi variant too (first call pays NEFF load)
    t_lo, t_hi = [], []
    for _ in range(iters):
        t_lo.append(call_lo(timed=True))
        t_hi.append(call_hi(timed=True))
    out = np.zeros((cfg.N, cfg.OUT), np.float32)
    for c in range(cfg.NC):
        op = results[c]["outperm"]
        perm = post["perms"][c]
        out[post["nodes_by_core"][c][perm[:cfg.OWN]]] = op[:cfg.OWN]
    return out, t_lo, t_hi


def kernel(**inputs):
    cfg = Cfg(N=50000, E=800000)
    out, _ = run(cfg, inputs, trace=False)
    return out


# revision 44
# speedup vs baseline: 1.0701x; 1.0701x over previous
"""2-layer GAT (DGL GATConv-style) on 8 Trainium2 NeuronCores.

v2 strategy (dst-sharded, matmul-direct layer 1):
  - Nodes are partitioned into 8 ranges by global degree rank (round-robin);
    core c owns its dsts and all edges pointing into them.  Own dsts are
    degree-sorted and tiled 128 at a time.
  - Layer 1 does NO gathering at all: the host pre-permutes x into per-core
    edge-slot order (xpair[:, (t,k,p)] = x[src of slot k of dst p in tile t]),
    so z1aug = xpair-block @ [W1 | W1@AL] lands in PSUM already in the
    [128 dsts, slot] layout the edge-softmax needs.  Pad slots use a host
    vector v with v@(W1@AL) = -100 so exp(lrelu(el+er)) vanishes.
  - Edge softmax: e = exp(lrelu(el_src + er_dst)), denominators via ACT
    accum, weighting + segment-sum via DVE binary-tree folds over slots.
  - Layer-2 projection z2aug = h @ [W2|W2@AL2|W2@AR2] from the layer-1 tiles
    (PE transpose), stored bf16 into a per-core shard in tile/perm order,
    AllGather -> tab2 (Shared), copied to a core-private table, then layer 2
    runs the dma_gather machinery: 256B bf16 rows [z2|el2,er2 f32], int16
    indices via two overlapping windows A=[0,32768) / B=[ROWS-32768,ROWS).
  - dma_gather throughput notes: descriptor GENERATION on the Q7 cluster is
    the floor (~4.2us per 1024-idx instruction, serial across the cluster);
    the drain is spread over 4 SWDGE queues (idx data must be replicated
    into partitions [32q,32q+32) for queue q -- see _pack_idx).  >1024 idxs
    per instruction overflows the SWDGE ring.
"""
import sys

sys.path.insert(0, "/opt/trn_rl_repo")

import numpy as np

import concourse.bass as bass
import concourse.mybir as mybir
import concourse.tile as tile_mod
from concourse import library_config
from concourse.library_overlay import lower_extended_insts
from concourse.tile import TileContext
from concourse.bass_utils import run_bass_kernel_spmd

F32 = mybir.dt.float32
BF16 = mybir.dt.bfloat16
I16 = mybir.dt.int16
AF = mybir.ActivationFunctionType
ALU = mybir.AluOpType

NEG_SLOPE = 0.2
SENT_EL = -1.0e30
L1C = 264          # g-slot cols (bf16): z(256) + el(4 f32 bitcast = 8)
L2C = 128          # tab2 row cols (bf16): z2(64) + el2,er2(f32 bitcast = 4)


# ---------------------------------------------------------------------------
# Workaround: this walrus build rejects Drain instructions with >1 sync wait.
def _patched_drain_and_barrier(self, tick_clock, wait_clock):
    nc = self.nc
    probe = nc.sync.drain()
    wait_clock.add_sem_waits(
        probe.ins, tile_mod.ScopedClock({None: tick_clock.global_clock})
    )
    si = probe.ins.sync_info
    waits = list(si.on_wait) if si is not None else []
    if len(waits) > 1:
        bb = nc.cur_bb.bb
        popped = bb.instructions.pop()
        assert popped is probe.ins
        by_name = {}
        for h in self.sems.allocated().values():
            by_name[h.name] = h
        for w in waits:
            assert w.wait_mode == "sem-ge-imm", w
            nc.sync.wait_ge(by_name[w.ant_name], w.wait_value)
        nc.sync.drain()
    nc.all_engine_barrier()
    popped_p = nc._tile_sem_poison_stack.pop()
    assert popped_p is self._sem_poison
    nc.clear_and_free_semaphores(list(self.sems.allocated().values()))
    nc.all_engine_barrier()


TileContext._drain_and_barrier = _patched_drain_and_barrier

_wsplit_n = 0


def _split_multi_waits(nc, keep=1):
    """This walrus build allows at most one sync-wait per instruction; hoist
    extra waits onto dedicated EventSemaphore instructions just before."""
    global _wsplit_n
    for f in nc.m.functions:
        for bb in f.blocks:
            need = any(
                inst.sync_info is not None and len(inst.sync_info.on_wait) > keep
                for inst in bb.instructions
            )
            if not need:
                continue
            newlist = []
            for inst in bb.instructions:
                si = inst.sync_info
                if si is not None and len(si.on_wait) > keep:
                    waits = list(si.on_wait)
                    for w in waits[:-keep]:
                        ev = mybir.InstEventSemaphore(
                            name=f"WSPLIT-{_wsplit_n}", ins=[], outs=[])
                        _wsplit_n += 1
                        ev.engine = inst.engine
                        ev.sync_info = mybir.SyncInfo(on_wait=[w], on_update=[])
                        newlist.append(ev)
                    inst.sync_info = mybir.SyncInfo(
                        on_wait=waits[-keep:], on_update=list(si.on_update))
                newlist.append(inst)
            try:
                bb.instructions[:] = newlist
            except TypeError:
                while len(bb.instructions):
                    bb.instructions.pop()
                for inst in newlist:
                    bb.instructions.append(inst)


# ---------------------------------------------------------------------------
def _pack_idx(logical):
    """int16 idx list -> [128, n/16] wrapped/replicated layout for dma_gather.

    Queue q's Q7 cpu pair reads idx data through SIMD channels
    [32q, 32q+32) -> SBUF partitions 32q..32q+31 of the idx tile, so the
    16-partition wrap is replicated 8x to cover all four queues."""
    n = len(logical)
    assert n % 16 == 0
    a = np.asarray(logical, np.int16).reshape(n // 16, 16).T
    out = np.empty((128, n // 16), np.int16)
    for b in range(8):
        out[16 * b:16 * (b + 1)] = a
    return out


class Cfg:
    def __init__(self, N, E, lim=32768):
        self.N = N
        self.E = E
        self.NC = 8
        self.IN = 256
        self.HID = 64
        self.H1 = 4
        self.OUT = 64
        self.OWN = N // self.NC
        self.OWNP = -(-self.OWN // 128) * 128
        self.ROWS = self.NC * self.OWNP
        self.LIM = lim                      # rows addressable by one window
        self.TBOFF = max(self.ROWS - lim, 0)  # start row of window B
        self.NT = self.OWNP // 128          # dst tiles per core
        self.SENT_A = self.OWN              # sentinel row (block 0 pad row)
        self.SENT_B = (self.NC - 1) * self.OWNP + self.OWN
        assert self.SENT_B < self.ROWS
        assert self.SENT_B - self.TBOFF < lim


def prep(cfg, x, W1, al1, ar1, b1, W2, al2, ar2, b2, src, dst):
    """Host-side graph partitioning / staging.  Returns (in_maps, sched, post)."""
    N, E, NC = cfg.N, cfg.E, cfg.NC
    IN, HID, H1, OUT = cfg.IN, cfg.HID, cfg.H1, cfg.OUT

    x = np.asarray(x, np.float32)
    src = np.asarray(src)
    dst = np.asarray(dst)
    W1 = np.asarray(W1, np.float32)
    W2 = np.asarray(W2, np.float32)
    al1 = np.asarray(al1, np.float32)
    ar1 = np.asarray(ar1, np.float32)
    al2 = np.asarray(al2, np.float32)
    ar2 = np.asarray(ar2, np.float32)
    b1 = np.asarray(b1, np.float32)
    b2 = np.asarray(b2, np.float32)

    # parameter transforms
    AL1 = np.zeros((H1 * HID, H1), np.float32)
    AR1 = np.zeros((H1 * HID, H1), np.float32)
    for h in range(H1):
        AL1[h * HID:(h + 1) * HID, h] = al1[h]
        AR1[h * HID:(h + 1) * HID, h] = ar1[h]
    W1aug = np.concatenate([W1, W1 @ AL1], axis=1)          # [IN, 260]
    W1ar = W1 @ AR1                                         # [IN, 4]
    AL2 = al2.reshape(OUT, 1)
    AR2 = ar2.reshape(OUT, 1)
    W2aug = np.concatenate([W2, W2 @ AL2, W2 @ AR2], axis=1)  # [256, 66]

    # pad-slot column: v @ (W1@AL1) = -100 (min-norm), so pad edges score
    # exp(lrelu(-100 + er)) ~ 0 while z_pad stays bounded.
    W1AL1 = W1 @ AL1
    v = np.linalg.lstsq(W1AL1.T, np.full(H1, -100.0, np.float32),
                        rcond=None)[0].astype(np.float32)

    bf = mybir.dt.np(BF16)
    xb = x.astype(bf)
    vb = v.astype(bf)
    el_pad = vb.astype(np.float32) @ W1AL1
    assert el_pad.max() < -50.0, el_pad

    # ownership: round-robin by global degree rank -> per-tile max degrees
    # align across cores (shared program, minimal padding)
    deg_g = np.bincount(dst, minlength=N)
    grank = np.argsort(-deg_g, kind="stable")
    owner = np.empty(N, np.int64)
    local_rank = np.empty(N, np.int64)
    owner[grank] = np.arange(N) % NC
    local_rank[grank] = np.arange(N) // NC
    nodes_by_core = [grank[c::NC] for c in range(NC)]   # local-rank order

    edge_by_core = [np.nonzero(owner[dst] == c)[0] for c in range(NC)]

    # per-core degree sort (tile order = perm order; shard is laid out in
    # perm order so tile outputs land contiguously -- no scatter needed).
    # Two passes: pass-0 perms from degree only -> provisional rows; pass-1
    # refines the tie-break with per-dst must-A counts under those rows.
    def _perms_to_rows(perms_):
        rows = np.empty(N, np.int64)
        for c in range(NC):
            inv = np.empty(cfg.OWN, np.int64)
            inv[perms_[c][:cfg.OWN]] = np.arange(cfg.OWN)
            rows[nodes_by_core[c]] = c * cfg.OWNP + inv
        return rows

    deg_by_core = []
    for c in range(NC):
        eid = edge_by_core[c]
        dloc = local_rank[dst[eid]]
        deg_by_core.append(np.bincount(dloc, minlength=cfg.OWN))

    def _mk_perms(rows_prov):
        perms_ = []
        for c in range(NC):
            eid = edge_by_core[c]
            dloc = local_rank[dst[eid]]
            if rows_prov is None:
                order = np.argsort(-deg_by_core[c], kind="stable")
            else:
                mA = np.bincount(dloc[rows_prov[src[eid]] < cfg.TBOFF],
                                 minlength=cfg.OWN)
                order = np.lexsort((mA, deg_by_core[c]))[::-1]
            perms_.append(np.concatenate(
                [order, np.full(cfg.OWNP - cfg.OWN, order[-1], np.int64)]))
        return perms_

    perms = _mk_perms(_perms_to_rows(_mk_perms(None)))

    # table row of node n = owner*OWNP + position of n in its core's perm
    row_of_node = _perms_to_rows(perms)
    rows_src = row_of_node[src]

    # per-core (src-rows, src-nodes) bucketed by local dst
    per_core_bydst = []
    for c in range(NC):
        eid = edge_by_core[c]
        dloc = local_rank[dst[eid]]
        so = np.argsort(dloc, kind="stable")
        srows_sorted = rows_src[eid][so]
        snode_sorted = src[eid][so]
        starts = np.searchsorted(dloc[so], np.arange(cfg.OWN + 1))
        per_core_bydst.append(
            [(srows_sorted[starts[i]:starts[i + 1]],
              snode_sorted[starts[i]:starts[i + 1]]) for i in range(cfg.OWN)])

    # ---- G1 schedule: per-tile slot count = max degree across cores ----
    K1s = []
    for t in range(cfg.NT):
        m = 1
        for c in range(NC):
            dts = perms[c][t * 128:(t + 1) * 128]
            hi = min((t + 1) * 128, cfg.OWN)
            if t * 128 < cfg.OWN:
                m = max(m, int(deg_by_core[c][dts[:hi - t * 128]].max()))
        K1s.append(m)
    E1 = sum(K1s) * 128

    # ---- G2 schedule: joint per-tile (alpha, beta) A/B window feasibility ----
    tiles_ab = []
    for c in range(NC):
        bydst = per_core_bydst[c]
        perm = perms[c]
        stats = []
        for t in range(cfg.NT):
            dts = perm[t * 128:(t + 1) * 128]
            mA = np.zeros(128, np.int64)
            mB = np.zeros(128, np.int64)
            dg = np.zeros(128, np.int64)
            for i in range(128):
                if t * 128 + i >= cfg.OWN:
                    continue
                rs = bydst[dts[i]][0]
                dg[i] = len(rs)
                mA[i] = int((rs < cfg.TBOFF).sum())
                mB[i] = int((rs >= cfg.LIM).sum())
            stats.append((mA, mB, dg))
        tiles_ab.append(stats)

    Ks = []
    for t in range(cfg.NT):
        alo = max(int(tiles_ab[c][t][0].max()) for c in range(NC))
        ahi = max(int(tiles_ab[c][t][2].max()) for c in range(NC))
        best = None
        for alpha in range(alo, ahi + 1):
            beta = 0
            for c in range(NC):
                mA, mB, dg = tiles_ab[c][t]
                beta = max(beta, int(np.maximum(
                    mB, dg - np.minimum(alpha, dg - mB)).max()))
            if best is None or alpha + beta < best[0] + best[1]:
                best = (alpha, beta)
        Ks.append((max(best[0], 1), max(best[1], 1)))

    # ---- per-core staging ----
    idx_all, xpairs, xTps = [], [], []
    for c in range(NC):
        perm = perms[c]
        bydst = per_core_bydst[c]
        # xpair: edge-slot-ordered x columns
        srcflat = np.full(E1, -1, np.int64)
        base = 0
        for t in range(cfg.NT):
            K = K1s[t]
            dts = perm[t * 128:(t + 1) * 128]
            for i in range(128):
                if t * 128 + i >= cfg.OWN:
                    continue
                nodes = bydst[dts[i]][1]
                srcflat[base + np.arange(len(nodes)) * 128 + i] = nodes
            base += K * 128
        xp = np.empty((E1, IN), bf)
        m = srcflat >= 0
        xp[m] = xb[srcflat[m]]
        xp[~m] = vb
        xpairs.append(np.ascontiguousarray(xp.T))

        # G2 idx tables (A/B windows)
        cols = []
        for t in range(cfg.NT):
            a_t, b_t = Ks[t]
            Aidx = np.full((a_t, 128), cfg.SENT_A, np.int64)
            Bidx = np.full((b_t, 128), cfg.SENT_B - cfg.TBOFF, np.int64)
            dts = perm[t * 128:(t + 1) * 128]
            for i in range(128):
                if t * 128 + i >= cfg.OWN:
                    continue
                rs = bydst[dts[i]][0]
                isA = rs < cfg.TBOFF
                isB = rs >= cfg.LIM
                flex = rs[~isA & ~isB]
                Alist = list(rs[isA])
                Blist = list(rs[isB])
                nA = min(a_t, len(Alist) + len(flex))
                take = nA - len(Alist)
                Alist += list(flex[:take])
                Blist += list(flex[take:])
                assert len(Alist) <= a_t and len(Blist) <= b_t, (t, i)
                Aidx[:len(Alist), i] = Alist
                Bidx[:len(Blist), i] = np.asarray(Blist) - cfg.TBOFF
            assert Aidx.max() < cfg.LIM
            cols.append(_pack_idx(Aidx.reshape(-1)))
            cols.append(_pack_idx(Bidx.reshape(-1)))
        idx_all.append(np.concatenate(cols, axis=1))
        xTps.append(np.ascontiguousarray(
            x[nodes_by_core[c][perm]].astype(bf).T))

    sent2 = np.zeros((1, L2C), bf)
    sent2.view(np.uint16)[0, 64:68] = np.array(
        [SENT_EL, SENT_EL], np.float32).view(np.uint16)

    b1bc = np.broadcast_to(b1.reshape(1, -1), (128, H1 * HID)).astype(bf)
    b2bc = np.broadcast_to(b2.reshape(1, -1), (128, OUT)).copy()
    ident = np.eye(128, dtype=bf)

    in_maps = []
    for c in range(NC):
        in_maps.append(
            {
                "xpair": xpairs[c],
                "xTp": xTps[c],
                "W1aug": W1aug.astype(bf),
                "W1ar": W1ar.astype(bf),
                "W2aug": W2aug.astype(bf),
                "b1bc": b1bc,
                "b2bc": b2bc,
                "ident": ident,
                "sent2": sent2,
                "idx_all": idx_all[c],
            }
        )
    sched = {"Ks": Ks, "K1s": K1s, "E1": E1, "idx_cols": idx_all[0].shape[1]}
    post = {"perms": perms, "nodes_by_core": nodes_by_core}
    return in_maps, sched, post


# ---------------------------------------------------------------------------
def build(cfg, sched, phases=4, g1_mode=5, g2_mode=1, reps=1, nq=4, chunk=8,
          g1b=4, g2b=6, zpb=2, tpb=1, xeb=3, grp=3, pairw=2, agch=1,
          shared_gather=False):
    Ks = sched["Ks"]
    K1s = sched["K1s"]
    E1 = sched["E1"]
    nc = bass.Bass(num_swdge_queues=nq)
    _qctr = [0]

    def nxq():
        q = _qctr[0] % nq
        _qctr[0] += 1
        return q
    IN, H1, HID, OUT = cfg.IN, cfg.H1, cfg.HID, cfg.OUT

    def P(name, shape, dt=F32):
        return nc.declare_dram_parameter(name, list(shape), dt, isOutput=False)

    xpair = P("xpair", [IN, E1], BF16)
    xTp = P("xTp", [IN, cfg.OWNP], BF16)
    W1a = P("W1aug", [IN, 260], BF16)
    W1r = P("W1ar", [IN, 4], BF16)
    W2a = P("W2aug", [IN, 66], BF16)
    b1b = P("b1bc", [128, 256], BF16)
    b2b = P("b2bc", [128, OUT])
    idn = P("ident", [128, 128], BF16)
    sent2 = P("sent2", [1, L2C], BF16)
    idx_all = P("idx_all", [128, sched["idx_cols"]], I16)
    outp = nc.declare_dram_parameter("outperm", [cfg.OWNP, OUT], F32, isOutput=True)

    shard = nc.dram_tensor("shard", [cfg.OWNP, L2C], BF16)
    tab2s = nc.dram_tensor("tab2s", [cfg.ROWS, L2C], BF16, addr_space="Shared")
    tab2st = nc.dram_tensor("tab2st", [cfg.ROWS, L2C], BF16, addr_space="Shared")
    if shared_gather:
        tab2 = tab2s
    else:
        tab2 = nc.dram_tensor("tab2", [cfg.ROWS, L2C], BF16)

    _regs = {}

    def nreg(v):
        if v not in _regs:
            _regs[v] = nc.gpsimd.to_reg(v)
        return _regs[v]

    with TileContext(nc) as tc:
        nc.gpsimd.load_library(library_config.mlp)
        with tc.tile_pool(name="const", bufs=1) as cp:
            w1a = cp.tile([128, 2 * 260], BF16, tag="w1a")
            w1r = cp.tile([128, 2 * 4], BF16, tag="w1r")
            w2a = cp.tile([128, 2 * 66], BF16, tag="w2a")
            b1s = cp.tile([128, 256], BF16, tag="b1s")
            b2s = cp.tile([128, OUT], F32, tag="b2s")
            ids = cp.tile([128, 128], BF16, tag="ids")
            er1 = cp.tile([128, cfg.NT * 4], F32, tag="er1")
            er2 = cp.tile([128, cfg.NT], F32, tag="er2")
            itall = cp.tile([128, sched["idx_cols"]], I16, tag="itall")
            nc.sync.dma_start(out=itall[:], in_=idx_all[:])
            for k in range(2):
                nc.sync.dma_start(out=w1a[:, k * 260:(k + 1) * 260],
                                  in_=W1a[k * 128:(k + 1) * 128, :])
                nc.sync.dma_start(out=w1r[:, k * 4:(k + 1) * 4],
                                  in_=W1r[k * 128:(k + 1) * 128, :])
                nc.sync.dma_start(out=w2a[:, k * 66:(k + 1) * 66],
                                  in_=W2a[k * 128:(k + 1) * 128, :])
            nc.sync.dma_start(out=b1s[:], in_=b1b[:])
            nc.sync.dma_start(out=b2s[:], in_=b2b[:])
            nc.sync.dma_start(out=ids[:], in_=idn[:])

            for _rep in range(reps):
                # ---------------- er1 mini-pass (own pool region) ----------
                ctxG1 = nc.named_scope("phaseG1"); ctxG1.__enter__()
                ESUP = 8
                with tc.tile_pool(name="ex1", bufs=2) as exp_, \
                     tc.tile_pool(name="ep1", bufs=2, space="PSUM") as epp:
                    for st in range(-(-cfg.NT // ESUP) if phases >= 1 else 0):
                        t0 = st * ESUP
                        ntl = min(ESUP, cfg.NT - t0)
                        cols = ntl * 128
                        xb = exp_.tile([128, 2, cols], BF16, tag="xb1")
                        for k in range(2):
                            nc.sync.dma_start(
                                out=xb[:, k, :],
                                in_=xTp[k * 128:(k + 1) * 128,
                                        t0 * 128:t0 * 128 + cols])
                        for i in range(ntl):
                            ep = epp.tile([128, 4], F32, tag="ep")
                            for k in range(2):
                                nc.tensor.matmul(
                                    ep[:], xb[:, k, i * 128:(i + 1) * 128],
                                    w1r[:, k * 4:(k + 1) * 4],
                                    start=(k == 0), stop=(k == 1))
                            nc.vector.tensor_copy(
                                er1[:, (t0 + i) * 4:(t0 + i) * 4 + 4], ep[:])

                # ---------------- phase G1: matmul-direct layer 1 ----------
                # 2-way software interleave: stages of tile pairs are emitted
                # alternately so each in-order engine queue always has an
                # independent chain behind a stalled wait.
                GRP = grp         # psum slots per group (bank-aligned slots)
                ecoffs = np.concatenate([[0], np.cumsum(np.asarray(K1s) * 128)])
                with tc.tile_pool(name="xe", bufs=xeb) as xep, \
                     tc.tile_pool(name="g1", bufs=g1b) as gp, \
                     tc.tile_pool(name="w1p", bufs=3) as wp, \
                     tc.tile_pool(name="hb", bufs=3) as hp, \
                     tc.tile_pool(name="s2", bufs=3) as s2p, \
                     tc.tile_pool(name="sm", bufs=4) as smp, \
                     tc.tile_pool(name="zp", bufs=zpb, space="PSUM") as zpp, \
                     tc.tile_pool(name="tp", bufs=tpb, space="PSUM") as tpp:

                    def g1_s1(t):
                        K = K1s[t]
                        ecoff = int(ecoffs[t])
                        xe = xep.tile([128, 2, K * 128], BF16, tag="xe")
                        for k in range(2):
                            nc.sync.dma_start(
                                out=xe[:, k, :],
                                in_=xpair[k * 128:(k + 1) * 128,
                                          ecoff:ecoff + K * 128])
                        g = gp.tile([128, K * L1C], BF16, tag="g")
                        ga = g[:]
                        gf = ga.bitcast(F32)
                        for g0 in range(0, K, GRP):
                            n = min(GRP, K - g0)
                            zpg = zpp.tile([128, GRP * 512], F32, tag="zpg")
                            for j in range(n):
                                for kb in range(2):
                                    nc.tensor.matmul(
                                        zpg[:, j * 512:j * 512 + 260],
                                        xe[:, kb, (g0 + j) * 128:(g0 + j + 1) * 128],
                                        w1a[:, kb * 260:(kb + 1) * 260],
                                        start=(kb == 0), stop=(kb == 1))
                            zpa = zpg[:]
                            # z cols: one strided ACT copy per group
                            nc.scalar.copy(
                                bass.AP(ga.tensor, ga.offset + g0 * L1C,
                                        [ga.ap[0], [L1C, n], [1, 256]]),
                                bass.AP(zpa.tensor, zpa.offset,
                                        [zpa.ap[0], [512, n], [1, 256]]))
                            # el cols: fused + er1 (one DVE add per group)
                            era = er1[:]
                            nc.vector.tensor_tensor(
                                bass.AP(gf.tensor,
                                        gf.offset + g0 * (L1C // 2) + 128,
                                        [gf.ap[0], [L1C // 2, n], [1, 4]]),
                                bass.AP(zpa.tensor, zpa.offset + 256,
                                        [zpa.ap[0], [512, n], [1, 4]]),
                                bass.AP(era.tensor, era.offset + 4 * t,
                                        [era.ap[0], [0, n], [1, 4]]),
                                op=ALU.add)
                        return dict(t=t, K=K, g=g, ga=ga, gf=gf)

                    def g1_s2(st):
                        if g1_mode < 1:
                            return
                        K, ga, gf = st["K"], st["ga"], st["gf"]
                        w = wp.tile([128, K * 4], BF16, tag="w")
                        s = smp.tile([128, 4], F32, tag="s")
                        rs = smp.tile([128, 4], F32, tag="rs")
                        # w laid out h-major: w[:, h*K + k]
                        el_all = bass.AP(gf.tensor, gf.offset + 128,
                                         [gf.ap[0], [L1C // 2, K], [1, 4]])
                        wa = w[:]
                        w_hm = bass.AP(wa.tensor, wa.offset,
                                       [wa.ap[0], [1, K], [K, 4]])
                        nc.scalar.activation(w_hm, el_all, AF.Prelu,
                                             scale=1.0, alpha=NEG_SLOPE)
                        for h in range(4):
                            nc.scalar.activation(
                                w[:, h * K:(h + 1) * K],
                                w[:, h * K:(h + 1) * K], AF.Exp,
                                accum_out=s[:, h:h + 1])
                        nc.vector.tensor_scalar_max(s[:], s[:], 1e-30)
                        nc.vector.reciprocal(rs[:], s[:])
                        st["w"], st["rs"] = w, rs

                    def g1_s3(st):
                        if g1_mode < 2:
                            return
                        K, ga = st["K"], st["ga"]
                        wa = st["w"][:]
                        gz = bass.AP(ga.tensor, ga.offset,
                                     [ga.ap[0], [L1C, K], [HID, 4], [1, HID]])
                        wbc = bass.AP(wa.tensor, wa.offset,
                                      [wa.ap[0], [1, K], [K, 4], [0, HID]])
                        nc.vector.tensor_tensor(gz, gz, wbc, op=ALU.mult)
                        Kc = K
                        while Kc > 1 and g1_mode >= 3:
                            half = Kc // 2
                            m = Kc - half
                            lo = bass.AP(ga.tensor, ga.offset,
                                         [ga.ap[0], [L1C, half], [1, 256]])
                            hi = bass.AP(ga.tensor, ga.offset + m * L1C,
                                         [ga.ap[0], [L1C, half], [1, 256]])
                            nc.vector.tensor_tensor(lo, lo, hi, op=ALU.add)
                            Kc = m

                    def g1_s4(st):
                        if g1_mode < 4:
                            return
                        t, ga, rs = st["t"], st["ga"], st["rs"]
                        hb = hp.tile([128, 256], BF16, tag="hb")
                        acc = bass.AP(ga.tensor, ga.offset,
                                      [ga.ap[0], [HID, 4], [1, HID]])
                        hba = hb[:]
                        hb4 = bass.AP(hba.tensor, hba.offset,
                                      [hba.ap[0], [HID, 4], [1, HID]])
                        rsa = rs[:]
                        rsb = bass.AP(rsa.tensor, rsa.offset,
                                      [rsa.ap[0], [1, 4], [0, HID]])
                        nc.vector.tensor_tensor(hb4, acc, rsb, op=ALU.mult)
                        nc.vector.tensor_tensor(hb[:], hb[:], b1s[:], op=ALU.add)
                        tmp = hp.tile([128, 256], BF16, tag="elutmp")
                        nc.vector.tensor_scalar_min(tmp[:], hb[:], 0.0)
                        nc.vector.tensor_scalar_max(hb[:], hb[:], 0.0)
                        nc.scalar.activation(tmp[:], tmp[:], AF.Exp)
                        nc.vector.tensor_tensor(hb[:], hb[:], tmp[:], op=ALU.add)
                        nc.vector.tensor_scalar_add(hb[:], hb[:], -1.0)
                        if g1_mode < 5:
                            return
                        zp2 = tpp.tile([128, 66], F32, tag="z2p")
                        for k in range(2):
                            tp = tpp.tile([128, 128], BF16, tag="tp")
                            nc.tensor.transpose(tp[:], hb[:, k * 128:(k + 1) * 128],
                                                ids[:])
                            hT = s2p.tile([128, 128], BF16, tag="hT")
                            nc.scalar.copy(hT[:], tp[:])
                            nc.tensor.matmul(zp2[:], hT[:],
                                             w2a[:, k * 66:(k + 1) * 66],
                                             start=(k == 0), stop=(k == 1))
                        z2sb = s2p.tile([128, L2C], BF16, tag="z2sb")
                        nc.vector.tensor_copy(er2[:, t:t + 1], zp2[:, 65:66])
                        nc.scalar.copy(z2sb[:, 0:64], zp2[:, 0:64])
                        nc.vector.tensor_copy(
                            z2sb.bitcast(F32)[:, 32:34], zp2[:, 64:66])
                        nc.scalar.dma_start(
                            out=shard[t * 128:(t + 1) * 128, :], in_=z2sb[:])

                    NT1 = cfg.NT if phases >= 1 else 0
                    for i in range(0, NT1, pairw):
                        ts = [t for t in range(i, i + pairw) if t < NT1]
                        sts = [g1_s1(t) for t in ts]
                        for st in sts:
                            g1_s2(st)
                        for st in sts:
                            g1_s3(st)
                        for st in sts:
                            g1_s4(st)
                    # pad-slot sentinel row (overwrites the last tile's write)
                    if phases >= 2:
                        nc.scalar.dma_start(
                            out=shard[cfg.OWN:cfg.OWN + 1, :], in_=sent2[:])

                # ---------------- allgather (+ local shadow copy) ----------
                ctxG1.__exit__(None, None, None)
                if phases >= 3:
                    with nc.named_scope("phaseAG"):
                        if agch <= 1:
                            nc.gpsimd.collective_compute(
                                "AllGather", ALU.bypass, ins=[shard[:]],
                                outs=[tab2s[:]],
                                replica_groups=[list(range(cfg.NC))])
                            if not shared_gather:
                                half = cfg.ROWS // 2
                                nc.sync.dma_start(out=tab2[0:half, :],
                                                  in_=tab2s[0:half, :])
                                nc.scalar.dma_start(out=tab2[half:, :],
                                                    in_=tab2s[half:, :])
                        else:
                            # chunked: AG chunk c overlaps copy of chunk c-1
                            # (chunk-major contiguous staging, copies strew
                            # rows into the core-major local table)
                            rows = cfg.OWNP // agch
                            rings = [nc.sync, nc.scalar]
                            for ci in range(agch):
                                s0 = ci * cfg.NC * rows
                                nc.gpsimd.collective_compute(
                                    "AllGather", ALU.bypass,
                                    ins=[shard[ci * rows:(ci + 1) * rows, :]],
                                    outs=[tab2st[s0:s0 + cfg.NC * rows, :]],
                                    replica_groups=[list(range(cfg.NC))])
                                for c in range(cfg.NC):
                                    r0 = c * cfg.OWNP + ci * rows
                                    rings[(ci * cfg.NC + c) % 2].dma_start(
                                        out=tab2[r0:r0 + rows, :],
                                        in_=tab2st[s0 + c * rows:
                                                   s0 + (c + 1) * rows, :])

                # ---------------- phase G2: layer 2 ----------------
                ioff = 0
                Kmax = max(a + b for a, b in Ks)
                ctxG2 = nc.named_scope("phaseG2"); ctxG2.__enter__()
                with tc.tile_pool(name="g2", bufs=g2b) as gp2, \
                     tc.tile_pool(name="gf2", bufs=2) as gfp, \
                     tc.tile_pool(name="w2p", bufs=2) as wp2, \
                     tc.tile_pool(name="ob", bufs=2) as op_, \
                     tc.tile_pool(name="sm2", bufs=4) as smp2:
                    for t in range(cfg.NT if phases >= 4 else 0):
                        a_t, b_t = Ks[t]
                        K = a_t + b_t
                        icols = K * 8
                        it = itall[:, ioff:ioff + icols]
                        g = gp2.tile([128, Kmax * L2C], BF16, tag="g2")
                        gv = g[:, :K * L2C].rearrange("p (k c) -> p k c", c=L2C)
                        for c0 in range(0, a_t, chunk):
                            n = min(chunk, a_t - c0)
                            nc.gpsimd.dma_gather(
                                out_ap=gv[:, c0:c0 + n, :], in_ap=tab2[:],
                                idxs_ap=it[:, c0 * 8:(c0 + n) * 8],
                                num_idxs=128 * n, queue_num=nxq(),
                                num_idxs_reg=nreg(128 * n), elem_size=L2C)
                        for c0 in range(0, b_t, chunk):
                            n = min(chunk, b_t - c0)
                            nc.gpsimd.dma_gather(
                                out_ap=gv[:, a_t + c0:a_t + c0 + n, :],
                                in_ap=tab2[cfg.TBOFF:, :],
                                idxs_ap=it[:, (a_t + c0) * 8:(a_t + c0 + n) * 8],
                                num_idxs=128 * n, queue_num=nxq(),
                                num_idxs_reg=nreg(128 * n), elem_size=L2C)
                        if g2_mode < 1:
                            ioff += icols
                            continue
                        w2t = wp2.tile([128, Kmax], F32, tag="w2t")
                        s2 = smp2.tile([128, 1], F32, tag="s2")
                        rs2 = smp2.tile([128, 1], F32, tag="rs2")
                        ga = g[:]
                        gaf = ga.bitcast(F32)
                        el2 = bass.AP(gaf.tensor, gaf.offset + 32,
                                      [gaf.ap[0], [L2C // 2, K]])
                        nc.scalar.activation(
                            w2t[:, :K], el2, AF.Prelu,
                            bias=er2[:, t:t + 1], scale=1.0, alpha=NEG_SLOPE)
                        nc.scalar.activation(
                            w2t[:, :K], w2t[:, :K], AF.Exp, accum_out=s2[:])
                        nc.vector.tensor_scalar_max(s2[:], s2[:], 1e-30)
                        nc.vector.reciprocal(rs2[:], s2[:])
                        wa = w2t[:]
                        gf2 = gfp.tile([128, Kmax * OUT], F32, tag="gf2")
                        g2fa = gf2[:]
                        gz = bass.AP(ga.tensor, ga.offset,
                                     [ga.ap[0], [L2C, K], [1, OUT]])
                        gzf = bass.AP(g2fa.tensor, g2fa.offset,
                                      [g2fa.ap[0], [OUT, K], [1, OUT]])
                        wbc = bass.AP(wa.tensor, wa.offset,
                                      [wa.ap[0], [1, K], [0, OUT]])
                        nc.vector.tensor_tensor(gzf, gz, wbc, op=ALU.mult)
                        Kc = K
                        while Kc > 1:
                            half = Kc // 2
                            m = Kc - half
                            lo = bass.AP(g2fa.tensor, g2fa.offset,
                                         [g2fa.ap[0], [OUT, half], [1, OUT]])
                            hi = bass.AP(g2fa.tensor, g2fa.offset + m * OUT,
                                         [g2fa.ap[0], [OUT, half], [1, OUT]])
                            nc.vector.tensor_tensor(lo, lo, hi, op=ALU.add)
                            Kc = m
                        ob = op_.tile([128, OUT], F32, tag="ob")
                        nc.vector.tensor_scalar_mul(ob[:], gf2[:, 0:OUT], rs2[:])
                        nc.vector.tensor_tensor(ob[:], ob[:], b2s[:], op=ALU.add)
                        nc.sync.dma_start(
                            out=outp[t * 128:(t + 1) * 128, :], in_=ob[:])
                        ioff += icols
                ctxG2.__exit__(None, None, None)

    _split_multi_waits(nc)
    lower_extended_insts(nc)
    return nc


# ---------------------------------------------------------------------------
_memo = {}


def run(cfg, inputs, trace=False, **bkw):
    in_maps, sched, post = prep(cfg, **inputs)
    key = (cfg.N, cfg.E, cfg.LIM, tuple(sched["Ks"]), tuple(sched["K1s"]),
           tuple(sorted(bkw.items())))
    if key not in _memo:
        _memo[key] = build(cfg, sched, **bkw)
    nc = _memo[key]
    res = run_bass_kernel_spmd(
        nc, in_maps, list(range(cfg.NC)), trace=trace)
    out = np.zeros((cfg.N, cfg.OUT), np.float32)
    for c in range(cfg.NC):
        op = res.results[c]["outperm"]
        perm = post["perms"][c]
        out[post["nodes_by_core"][c][perm[:cfg.OWN]]] = op[:cfg.OWN]
    return out, res


def _make_exec(cfg, nc, in_maps):
    """Compile nc into a donated sharded callable; returns (call, decode)."""
    import jax
    from jax.experimental.shard_map import shard_map
    from jax.sharding import Mesh, PartitionSpec

    from concourse import bass2jax

    bass2jax.install_neuronx_cc_hook()
    partition_name = nc.partition_id_tensor.name if nc.partition_id_tensor else None
    in_names, out_names, out_avals, zero_outs = [], [], [], []
    for alloc in nc.m.functions[0].allocations:
        if not isinstance(alloc, mybir.MemoryLocationSet):
            continue
        name = alloc.memorylocations[0].name
        if alloc.kind == "ExternalInput":
            if name != partition_name:
                in_names.append(name)
        elif alloc.kind == "ExternalOutput":
            out_names.append(name)
            shape = tuple(alloc.tensor_shape)
            dtype = mybir.dt.np(alloc.dtype)
            out_avals.append(jax.core.ShapedArray(shape, dtype))
            zero_outs.append(np.zeros(shape, dtype))
    n_params = len(in_names)
    n_outs = len(out_avals)
    all_in_names = list(in_names) + list(out_names)
    if partition_name is not None:
        all_in_names.append(partition_name)
    donate = tuple(range(n_params, n_params + n_outs))

    def _body(*args):
        operands = list(args)
        if partition_name is not None:
            operands.append(bass2jax.partition_id_tensor())
        outs = bass2jax._bass_exec_p.bind(
            *operands,
            out_avals=tuple(out_avals),
            in_names=tuple(all_in_names),
            out_names=tuple(out_names),
            lowering_input_output_aliases=(),
            sim_require_finite=True,
            sim_require_nnan=True,
            nc=nc,
        )
        return tuple(outs)

    NCOR = cfg.NC
    devices = jax.devices()[:NCOR]
    mesh = Mesh(np.asarray(devices), ("core",))
    in_specs = (PartitionSpec("core"),) * (n_params + n_outs)
    out_specs = (PartitionSpec("core"),) * len(out_names)
    sharded = jax.jit(
        shard_map(_body, mesh=mesh, in_specs=in_specs, out_specs=out_specs,
                  check_rep=False),
        donate_argnums=donate, keep_unused=True)
    sharding = jax.sharding.NamedSharding(mesh, PartitionSpec("core"))
    concat_in = [
        jax.device_put(
            np.concatenate([np.asarray(in_maps[c][n]) for c in range(NCOR)],
                           axis=0), sharding)
        for n in in_names
    ]

    def fresh_zeros():
        return [
            jax.device_put(
                np.zeros((NCOR * z.shape[0], *z.shape[1:]), z.dtype), sharding)
            for z in zero_outs
        ]

    def call(timed=False):
        import time
        zs = fresh_zeros()
        jax.block_until_ready(zs)
        t0 = time.perf_counter()
        o = sharded(*concat_in, *zs)
        jax.block_until_ready(o)
        return (time.perf_counter() - t0) if timed else o

    def decode(out_arrs):
        return [
            {n: np.asarray(out_arrs[i]).reshape(NCOR, *out_avals[i].shape)[c]
             for i, n in enumerate(out_names)}
            for c in range(NCOR)
        ]

    return call, decode


def _nc_for(cfg, sched, reps, **bkw):
    key = (cfg.N, cfg.E, cfg.LIM, tuple(sched["Ks"]), tuple(sched["K1s"]),
           reps, tuple(sorted(bkw.items())))
    if key not in _memo:
        _memo[key] = build(cfg, sched, reps=reps, **bkw)
    return _memo[key]


def run_bench_pair(cfg, inputs, iters=10, reps_lo=1, reps_hi=11, **bkw):
    """Correctness output plus interleaved wall-time samples of a reps_lo and
    a reps_hi NEFF.  Interleaving makes both variants see the same host/
    tunnel conditions, so min(hi)-min(lo) differences out the fixed dispatch
    overhead and isolates hardware execution time."""
    prepped = prep(cfg, **inputs)
    in_maps, sched, post = prepped
    call_lo, decode = _make_exec(cfg, _nc_for(cfg, sched, reps_lo, **bkw), in_maps)
    call_hi, _ = _make_exec(cfg, _nc_for(cfg, sched, reps_hi, **bkw), in_maps)
    results = decode(call_lo())
    call_hi()  # warm the hi variant too (first call pays NEFF load)
    t_lo, t_hi = [], []
    for _ in range(iters):
        t_lo.append(call_lo(timed=True))
        t_hi.append(call_hi(timed=True))
    out = np.zeros((cfg.N, cfg.OUT), np.float32)
    for c in range(cfg.NC):
        op = results[c]["outperm"]
        perm = post["perms"][c]
        out[post["nodes_by_core"][c][perm[:cfg.OWN]]] = op[:cfg.OWN]
    return out, t_lo, t_hi


def kernel(**inputs):
    cfg = Cfg(N=50000, E=800000)
    out, _ = run(cfg, inputs, trace=False)
    return out
